# revision 9
# baseline (speedup 1.0000x reference)
"""DeepSeek-V3 MoE (T=4096, H=2048, E=32 top-8/32 grouped, I=1024, IS=2048)
on 8 trn2 NeuronCores — self-contained kernel.

Strategy (expert-parallel + token dispatch per the sharding hint):
- Routing (gate gemm + sigmoid + grouped top-k) runs on host in fp32:
  0.03% of FLOPs, but expert SELECTION must match the fp32 reference.
- Experts are sorted by token count and grouped into EL=4 slots of 8;
  core c runs the c-th expert of each slot. All cores execute the same
  program, so per-slot capacity = max token count in the slot (rounded
  up to 32) — near-exact, ~2% padding vs 25% for a fixed capacity.
- Gemms run in bf16 (fp32 PSUM accumulate): same PE rate as fp32r but
  half the HBM traffic, so weight streaming stays far under the PE.
- Phase C accumulates each PSUM block over all 16 h-steps back-to-back
  (sequential per block) so a bank is drained ~10us before reuse.
- Combine weights are folded in on the host during the scatter-add, so
  the device returns raw per-expert outputs (fp32).
- The shared expert runs at full intermediate width over this core's
  T/8 token slice (token-parallel: outputs disjoint, no all-reduce).
"""
import contextlib
import numpy as np
import ml_dtypes

import concourse.bass as bass
import concourse.mybir as mybir
import concourse.tile as tile
from concourse import bacc

F32 = mybir.dt.float32
BF16 = mybir.dt.bfloat16
AF = mybir.ActivationFunctionType
NPBF16 = ml_dtypes.bfloat16

TOP_K, N_GROUP, TOPK_GROUP, ROUTED_SCALE = 8, 8, 4, 2.5
T, H, E, I, IS = 4096, 2048, 32, 1024, 2048
N_CORES = 8
EL = E // N_CORES          # 4 slot sections in the program
TSH = T // N_CORES         # 512-token shared slice per core
HT = H // 128              # 16
IT = I // 128              # 8
IST = IS // 128            # 16 i-tiles of the full shared intermediate
CAP_ALIGN = 32


def host_routing(x, gate_w, e_bias):
    """fp32 numpy replica of reference _routing_weights -> dense [T, E]."""
    logits = (x @ gate_w.T).astype(np.float32)
    scores = (1.0 / (1.0 + np.exp(-logits.astype(np.float32)))).astype(np.float32)
    swb = scores + e_bias[None, :].astype(np.float32)
    t, e = swb.shape
    gsz = e // N_GROUP
    grp = swb.reshape(t, N_GROUP, gsz)
    # top-2 sum per group (values only; ties irrelevant for a sum)
    top2 = np.sort(grp, axis=-1)[:, :, -2:]
    gscores = top2.sum(-1, dtype=np.float32)
    # top TOPK_GROUP groups, lowest-index-first on ties like jax.lax.top_k
    gidx = np.argsort(-gscores, axis=-1, kind="stable")[:, :TOPK_GROUP]
    gmask = np.zeros((t, N_GROUP), bool)
    np.put_along_axis(gmask, gidx, True, axis=1)
    emask = np.repeat(gmask, gsz, axis=1)
    masked = np.where(emask, swb, -np.inf)
    idx = np.argsort(-masked, axis=-1, kind="stable")[:, :TOP_K]
    w = np.take_along_axis(scores, idx, axis=1)
    w = (w / (w.sum(-1, keepdims=True) + 1e-20) * ROUTED_SCALE).astype(np.float32)
    wfull = np.zeros((t, e), np.float32)
    np.put_along_axis(wfull, idx, w, axis=1)
    return wfull


def plan_slots(wfull):
    """Group the E experts into EL slots of N_CORES by token count so that
    per-slot capacity (max count in slot, aligned) hugs the real counts."""
    counts = (wfull != 0).sum(0)
    order = np.argsort(-counts, kind="stable")
    slots = [order[N_CORES * j : N_CORES * (j + 1)] for j in range(EL)]
    caps = []
    for j in range(EL):
        m = int(counts[slots[j]].max())
        caps.append(max(256, -(-m // CAP_ALIGN) * CAP_ALIGN))
    return slots, tuple(caps)


def blocks_of(cap):
    """[(offset, size), ...]: near-equal blocks of 256..512 covering cap."""
    k = -(-cap // 512)
    base, rem = divmod(cap, k)
    sizes = [base + 1] * rem + [base] * (k - rem)
    out, off = [], 0
    for s in sizes:
        out.append((off, s))
        off += s
    return out


def build_nc2(caps, repeat=1, tsh=TSH):
    caps = tuple(int(c) for c in caps)
    total_cap = sum(caps)
    nc = bacc.Bacc("TRN2", target_bir_lowering=False)

    xs_d = nc.dram_tensor("xs", [H, total_cap], BF16, kind="ExternalInput")
    xsh_d = nc.dram_tensor("xsh", [H, tsh], BF16, kind="ExternalInput")
    w13_d = nc.dram_tensor("w13", [EL, IT, H, 256], BF16, kind="ExternalInput")
    w2t_d = nc.dram_tensor("w2t", [EL, I, H], BF16, kind="ExternalInput")
    sw13_d = nc.dram_tensor("sw13", [IST, H, 256], BF16, kind="ExternalInput")
    sw2t_d = nc.dram_tensor("sw2t", [IS, H], BF16, kind="ExternalInput")
    yd_d = nc.dram_tensor("yd", [H, total_cap], F32, kind="ExternalOutput")
    ys_d = nc.dram_tensor("ys", [H, tsh], F32, kind="ExternalOutput")

    with tile.TileContext(nc) as tc:
        with (
            tc.tile_pool(name="xp", bufs=1) as xp,
            tc.tile_pool(name="wp", bufs=3) as wp,
            tc.tile_pool(name="w2p", bufs=3) as w2p,
            tc.tile_pool(name="hp", bufs=1) as hp,
            tc.tile_pool(name="sp", bufs=1) as sp,
            tc.tile_pool(name="ps", bufs=1, space="PSUM") as ps,
        ):
            rep = tc.For_i(0, repeat, 1) if repeat > 1 else contextlib.nullcontext()
            with rep:
                # ============ routed experts over dispatched tokens
                for j in range(EL):
                    cape = caps[j]
                    s0 = sum(caps[:j])
                    blks = blocks_of(cape)
                    x_sb = xp.tile([128, HT * cape], BF16, tag="x", name="x_sb")
                    nc.sync.dma_start(
                        x_sb[:].rearrange("p (h t) -> p h t", h=HT),
                        xs_d[:, s0 : s0 + cape].rearrange(
                            "(h p) t -> p h t", p=128))

                    h_sb = hp.tile([128, IT * cape], BF16, tag="h", name="h_sb")
                    # ---- phase C: h = silu(w1@x) * (w3@x)
                    for ig in range(IT):
                        w13_sb = wp.tile([128, HT * 256], BF16, tag="w13",
                                         name="w13_sb")
                        nc.sync.dma_start(
                            w13_sb[:].rearrange("p (h c) -> p h c", h=HT),
                            w13_d[j, ig].rearrange("(h p) c -> p h c", p=128))
                        g_ps, u_ps = [], []
                        for gu in range(2):
                            for b, (bo, bs) in enumerate(blks):
                                p = ps.tile([128, bs], F32,
                                            tag=f"{'gu'[gu]}{b % 3}",
                                            name=f"{'gu'[gu]}_ps{b % 3}")
                                (g_ps if gu == 0 else u_ps).append(p)
                                for h in range(HT):
                                    nc.tensor.matmul(
                                        p[:],
                                        w13_sb[:, h * 256 + gu * 128
                                               : h * 256 + gu * 128 + 128],
                                        x_sb[:, h * cape + bo
                                             : h * cape + bo + bs],
                                        start=(h == 0), stop=(h == HT - 1))
                        for b, (bo, bs) in enumerate(blks):
                            silu_sb = sp.tile([128, 512], BF16, tag="silu",
                                              bufs=4, name="silu_sb")
                            nc.scalar.activation(silu_sb[:, :bs], g_ps[b][:],
                                                 AF.Silu)
                            nc.vector.tensor_mul(
                                h_sb[:, ig * cape + bo : ig * cape + bo + bs],
                                u_ps[b][:], silu_sb[:, :bs])

                    # ---- phase D: yd = w2 @ h
                    for hg in range(HT // 2):
                        w2_sb = w2p.tile([128, IT * 256], BF16, tag="w2",
                                         name="w2_sb")
                        nc.sync.dma_start(
                            w2_sb[:].rearrange("p (i c) -> p i c", i=IT),
                            w2t_d[j, :, 256 * hg : 256 * (hg + 1)].rearrange(
                                "(i p) c -> p i c", p=128))
                        for hl in range(2):
                            hrow = 256 * hg + 128 * hl
                            ost = sp.tile([128, cape], F32, tag="ost",
                                          bufs=4, name="ost")
                            for b, (bo, bs) in enumerate(blks):
                                o_ps = ps.tile([128, bs], F32, tag="o", bufs=2,
                                               name="o_ps")
                                for i in range(IT):
                                    nc.tensor.matmul(
                                        o_ps[:],
                                        w2_sb[:, i * 256 + hl * 128
                                              : i * 256 + hl * 128 + 128],
                                        h_sb[:, i * cape + bo
                                             : i * cape + bo + bs],
                                        start=(i == 0), stop=(i == IT - 1))
                                nc.vector.tensor_copy(ost[:, bo : bo + bs],
                                                      o_ps[:])
                            nc.sync.dma_start(
                                yd_d[hrow : hrow + 128, s0 : s0 + cape],
                                ost[:])

                # ============ shared expert, full IS, this core's 512 tokens
                x_sb = xp.tile([128, HT * tsh], BF16, tag="x", name="xsh_sb")
                nc.sync.dma_start(
                    x_sb[:].rearrange("p (h t) -> p h t", h=HT),
                    xsh_d[:].rearrange("(h p) t -> p h t", p=128))
                hs_sb = hp.tile([128, IST * tsh], BF16, tag="h", name="hs_sb")
                for ig in range(IST):
                    w13_sb = wp.tile([128, HT * 256], BF16, tag="w13",
                                     name="sw13_sb")
                    nc.sync.dma_start(
                        w13_sb[:].rearrange("p (h c) -> p h c", h=HT),
                        sw13_d[ig].rearrange("(h p) c -> p h c", p=128))
                    g_ps = ps.tile([128, tsh], F32, tag=f"g{ig % 2}",
                                   name="g_ps0")
                    u_ps = ps.tile([128, tsh], F32, tag=f"u{ig % 2}",
                                   name="u_ps0")
                    for gu in range(2):
                        p = g_ps if gu == 0 else u_ps
                        for h in range(HT):
                            nc.tensor.matmul(
                                p[:],
                                w13_sb[:, h * 256 + gu * 128
                                       : h * 256 + gu * 128 + 128],
                                x_sb[:, bass.ts(h, tsh)],
                                start=(h == 0), stop=(h == HT - 1))
                    silu_sb = sp.tile([128, 512], BF16, tag="silu", bufs=4,
                                      name="silu_sb")
                    nc.scalar.activation(silu_sb[:, :tsh], g_ps[:], AF.Silu)
                    nc.vector.tensor_mul(hs_sb[:, bass.ts(ig, tsh)], u_ps[:],
                                         silu_sb[:, :tsh])
                for hg in range(HT // 2):
                    w2_sb = w2p.tile([128, IST * 256], BF16, tag="w2",
                                     name="sw2_sb")
                    nc.sync.dma_start(
                        w2_sb[:].rearrange("p (i c) -> p i c", i=IST),
                        sw2t_d[:, 256 * hg : 256 * (hg + 1)].rearrange(
                            "(i p) c -> p i c", p=128))
                    for hl in range(2):
                        hrow = 256 * hg + 128 * hl
                        o_ps = ps.tile([128, tsh], F32, tag="o", bufs=2,
                                       name="o_ps")
                        for i in range(IST):
                            nc.tensor.matmul(
                                o_ps[:],
                                w2_sb[:, i * 256 + hl * 128
                                      : i * 256 + hl * 128 + 128],
                                hs_sb[:, bass.ts(i, tsh)],
                                start=(i == 0), stop=(i == IST - 1))
                        ost = sp.tile([128, 512], F32, tag="ost", bufs=4,
                                      name="ost")
                        nc.vector.tensor_copy(ost[:, :tsh], o_ps[:])
                        nc.sync.dma_start(ys_d[hrow : hrow + 128, :],
                                          ost[:, :tsh])
    nc.compile()
    return nc


def prep_inputs2(hidden_states, gate_w, e_bias, w1, w3, w2, sw1, sw3, sw2,
                 slots=None, caps=None):
    x = np.asarray(hidden_states, np.float32)
    t_total = x.shape[0]
    tsh = t_total // N_CORES
    wfull = host_routing(x, np.asarray(gate_w, np.float32),
                         np.asarray(e_bias, np.float32))
    if slots is None:
        slots, caps = plan_slots(wfull)
    total_cap = sum(caps)
    xT = np.ascontiguousarray(x.T.astype(NPBF16))

    w1t = np.asarray(w1, np.float32).transpose(0, 2, 1).astype(NPBF16)
    w3t = np.asarray(w3, np.float32).transpose(0, 2, 1).astype(NPBF16)
    w13 = np.empty((E, H, IT, 256), NPBF16)
    w13[..., 0:128] = np.ascontiguousarray(w1t).reshape(E, H, IT, 128)
    w13[..., 128:256] = np.ascontiguousarray(w3t).reshape(E, H, IT, 128)
    w13 = np.ascontiguousarray(w13.transpose(0, 2, 1, 3))  # [E, IT, H, 256]
    w2t = np.ascontiguousarray(
        np.asarray(w2, np.float32).transpose(0, 2, 1).astype(NPBF16))

    sw1t = np.ascontiguousarray(np.asarray(sw1, np.float32).T)  # [H, IS]
    sw3t = np.ascontiguousarray(np.asarray(sw3, np.float32).T)
    sw13 = np.empty((H, IST, 256), NPBF16)
    for ig in range(IST):
        sw13[:, ig, 0:128] = sw1t[:, 128 * ig : 128 * (ig + 1)]
        sw13[:, ig, 128:256] = sw3t[:, 128 * ig : 128 * (ig + 1)]
    sw13 = np.ascontiguousarray(sw13.transpose(1, 0, 2))  # [IST, H, 256]
    sw2t = np.ascontiguousarray(
        np.asarray(sw2, np.float32).T.astype(NPBF16))  # [IS, H]

    in_maps, scat = [], []
    for c in range(N_CORES):
        idxs, toks = [], []
        for j in range(EL):
            e_id = int(slots[j][c])
            idx = np.nonzero(wfull[:, e_id])[0]
            assert len(idx) <= caps[j], f"expert {e_id}: {len(idx)} > {caps[j]}"
            pad = caps[j] - len(idx)
            toks.append(np.concatenate([idx, np.zeros(pad, np.int64)]))
            idxs.append((e_id, idx))
        tok_list = np.concatenate(toks)
        in_maps.append({
            "xs": np.ascontiguousarray(xT[:, tok_list]),
            "xsh": np.ascontiguousarray(xT[:, c * tsh : (c + 1) * tsh]),
            "w13": np.ascontiguousarray(w13[[slots[j][c] for j in range(EL)]]),
            "w2t": np.ascontiguousarray(w2t[[slots[j][c] for j in range(EL)]]),
            "sw13": sw13, "sw2t": sw2t,
        })
        scat.append(idxs)
    return in_maps, scat, wfull, caps


def combine2(results, scat, wfull, caps, t_total=T):
    tsh = t_total // N_CORES
    acc = np.zeros((H, t_total), np.float32)
    for c in range(N_CORES):
        acc[:, c * tsh : (c + 1) * tsh] = results[c]["ys"]
    for c in range(N_CORES):
        yd = results[c]["yd"]
        for j, (e_id, idx) in enumerate(scat[c]):
            s0 = sum(caps[:j])
            colv = wfull[idx, e_id].astype(np.float32)
            acc[:, idx] += yd[:, s0 : s0 + len(idx)] * colv[None, :]
    return np.ascontiguousarray(acc.T)


_NC_CACHE = {}


def run2(inputs):
    from concourse.bass_utils import run_bass_kernel_spmd
    x = np.asarray(inputs["hidden_states"], np.float32)
    wfull = host_routing(x, np.asarray(inputs["gate_w"], np.float32),
                         np.asarray(inputs["e_bias"], np.float32))
    slots, caps = plan_slots(wfull)
    key = (caps, x.shape[0])
    if key not in _NC_CACHE:
        _NC_CACHE[key] = build_nc2(caps, tsh=x.shape[0] // N_CORES)
    nc = _NC_CACHE[key]
    in_maps, scat, wfull, caps = prep_inputs2(**inputs, slots=slots, caps=caps)
    res = run_bass_kernel_spmd(nc, in_maps, core_ids=list(range(N_CORES)))
    return combine2(res.results, scat, wfull, caps, t_total=x.shape[0]), res


def kernel(**inputs) -> np.ndarray:
    out, _ = run2(inputs)
    return np.asarray(out, np.float32)


# revision 11
# speedup vs baseline: 1.0141x; 1.0141x over previous
"""DeepSeek-V3 MoE (T=4096, H=2048, E=32 top-8/32 grouped, I=1024, IS=2048)
on 8 trn2 NeuronCores — self-contained kernel.

Strategy (expert-parallel + token dispatch per the sharding hint):
- Routing (gate gemm + sigmoid + grouped top-k) runs on host in fp32:
  0.03% of FLOPs, but expert SELECTION must match the fp32 reference.
- Experts are sorted by token count and grouped into EL=4 slots of 8;
  core c runs the c-th expert of each slot. All cores execute the same
  program, so per-slot capacity = max token count in the slot (rounded
  up to 32) — near-exact, ~2% padding vs 25% for a fixed capacity.
- Gemms run in bf16 (fp32 PSUM accumulate): same PE rate as fp32r but
  half the HBM traffic, so weight streaming stays far under the PE.
- Phase C accumulates each PSUM block over all 16 h-steps back-to-back
  (sequential per block) so a bank is drained ~10us before reuse.
- Combine weights are folded in on the host during the scatter-add, so
  the device returns raw per-expert outputs (fp32).
- The shared expert runs at full intermediate width over this core's
  T/8 token slice (token-parallel: outputs disjoint, no all-reduce).
"""
import contextlib
import numpy as np
import ml_dtypes

import concourse.bass as bass
import concourse.mybir as mybir
import concourse.tile as tile
from concourse import bacc

F32 = mybir.dt.float32
BF16 = mybir.dt.bfloat16
AF = mybir.ActivationFunctionType
NPBF16 = ml_dtypes.bfloat16

TOP_K, N_GROUP, TOPK_GROUP, ROUTED_SCALE = 8, 8, 4, 2.5
T, H, E, I, IS = 4096, 2048, 32, 1024, 2048
N_CORES = 8
EL = E // N_CORES          # 4 slot sections in the program
TSH = T // N_CORES         # 512-token shared slice per core
HT = H // 128              # 16
IT = I // 128              # 8
IST = IS // 128            # 16 i-tiles of the full shared intermediate
CAP_ALIGN = 32


def host_routing(x, gate_w, e_bias):
    """fp32 numpy replica of reference _routing_weights -> dense [T, E]."""
    logits = (x @ gate_w.T).astype(np.float32)
    scores = (1.0 / (1.0 + np.exp(-logits.astype(np.float32)))).astype(np.float32)
    swb = scores + e_bias[None, :].astype(np.float32)
    t, e = swb.shape
    gsz = e // N_GROUP
    grp = swb.reshape(t, N_GROUP, gsz)
    # top-2 sum per group (values only; ties irrelevant for a sum)
    top2 = np.sort(grp, axis=-1)[:, :, -2:]
    gscores = top2.sum(-1, dtype=np.float32)
    # top TOPK_GROUP groups, lowest-index-first on ties like jax.lax.top_k
    gidx = np.argsort(-gscores, axis=-1, kind="stable")[:, :TOPK_GROUP]
    gmask = np.zeros((t, N_GROUP), bool)
    np.put_along_axis(gmask, gidx, True, axis=1)
    emask = np.repeat(gmask, gsz, axis=1)
    masked = np.where(emask, swb, -np.inf)
    idx = np.argsort(-masked, axis=-1, kind="stable")[:, :TOP_K]
    w = np.take_along_axis(scores, idx, axis=1)
    w = (w / (w.sum(-1, keepdims=True) + 1e-20) * ROUTED_SCALE).astype(np.float32)
    wfull = np.zeros((t, e), np.float32)
    np.put_along_axis(wfull, idx, w, axis=1)
    return wfull


def plan_slots(wfull):
    """Group the E experts into EL slots of N_CORES by token count so that
    per-slot capacity (max count in slot, aligned) hugs the real counts."""
    counts = (wfull != 0).sum(0)
    order = np.argsort(-counts, kind="stable")
    slots = [order[N_CORES * j : N_CORES * (j + 1)] for j in range(EL)]
    caps = []
    for j in range(EL):
        m = int(counts[slots[j]].max())
        caps.append(max(256, -(-m // CAP_ALIGN) * CAP_ALIGN))
    return slots, tuple(caps)


def blocks_of(cap):
    """[(offset, size), ...]: near-equal blocks of 256..512 covering cap."""
    k = -(-cap // 512)
    base, rem = divmod(cap, k)
    sizes = [base + 1] * rem + [base] * (k - rem)
    out, off = [], 0
    for s in sizes:
        out.append((off, s))
        off += s
    return out


def build_nc2(caps, repeat=1, tsh=TSH):
    caps = tuple(int(c) for c in caps)
    total_cap = sum(caps)
    nc = bacc.Bacc("TRN2", target_bir_lowering=False)

    xs_d = nc.dram_tensor("xs", [128, HT * total_cap], BF16,
                          kind="ExternalInput")
    xsh_d = nc.dram_tensor("xsh", [128, HT * tsh], BF16, kind="ExternalInput")
    w13_d = nc.dram_tensor("w13", [EL, IT, 128, HT * 256], BF16,
                           kind="ExternalInput")
    w2t_d = nc.dram_tensor("w2t", [EL, HT // 2, 128, IT * 256], BF16,
                           kind="ExternalInput")
    sw13_d = nc.dram_tensor("sw13", [IST, 128, HT * 256], BF16,
                            kind="ExternalInput")
    sw2t_d = nc.dram_tensor("sw2t", [HT // 2, 128, IST * 256], BF16,
                            kind="ExternalInput")
    yd_d = nc.dram_tensor("yd", [H, total_cap], F32, kind="ExternalOutput")
    ys_d = nc.dram_tensor("ys", [H, tsh], F32, kind="ExternalOutput")

    with tile.TileContext(nc) as tc:
        with (
            tc.tile_pool(name="xp", bufs=1) as xp,
            tc.tile_pool(name="wp", bufs=3) as wp,
            tc.tile_pool(name="w2p", bufs=3) as w2p,
            tc.tile_pool(name="hp", bufs=1) as hp,
            tc.tile_pool(name="sp", bufs=1) as sp,
            tc.tile_pool(name="ps", bufs=1, space="PSUM") as ps,
        ):
            rep = tc.For_i(0, repeat, 1) if repeat > 1 else contextlib.nullcontext()
            with rep:
                # ============ routed experts over dispatched tokens
                for j in range(EL):
                    cape = caps[j]
                    s0 = sum(caps[:j])
                    blks = blocks_of(cape)
                    x_sb = xp.tile([128, HT * cape], BF16, tag="x", name="x_sb")
                    nc.sync.dma_start(
                        x_sb[:], xs_d[:, HT * s0 : HT * (s0 + cape)])

                    h_sb = hp.tile([128, IT * cape], BF16, tag="h", name="h_sb")
                    # ---- phase C: h = silu(w1@x) * (w3@x)
                    for ig in range(IT):
                        w13_sb = wp.tile([128, HT * 256], BF16, tag="w13",
                                         name="w13_sb")
                        nc.sync.dma_start(w13_sb[:], w13_d[j, ig])
                        g_ps, u_ps = [], []
                        for gu in range(2):
                            for b, (bo, bs) in enumerate(blks):
                                p = ps.tile([128, bs], F32,
                                            tag=f"{'gu'[gu]}{b % 3}",
                                            name=f"{'gu'[gu]}_ps{b % 3}")
                                (g_ps if gu == 0 else u_ps).append(p)
                                for h in range(HT):
                                    nc.tensor.matmul(
                                        p[:],
                                        w13_sb[:, h * 256 + gu * 128
                                               : h * 256 + gu * 128 + 128],
                                        x_sb[:, h * cape + bo
                                             : h * cape + bo + bs],
                                        start=(h == 0), stop=(h == HT - 1))
                        for b, (bo, bs) in enumerate(blks):
                            silu_sb = sp.tile([128, 512], BF16, tag="silu",
                                              bufs=4, name="silu_sb")
                            nc.scalar.activation(silu_sb[:, :bs], g_ps[b][:],
                                                 AF.Silu)
                            nc.vector.tensor_mul(
                                h_sb[:, ig * cape + bo : ig * cape + bo + bs],
                                u_ps[b][:], silu_sb[:, :bs])

                    # ---- phase D: yd = w2 @ h
                    for hg in range(HT // 2):
                        w2_sb = w2p.tile([128, IT * 256], BF16, tag="w2",
                                         name="w2_sb")
                        nc.sync.dma_start(w2_sb[:], w2t_d[j, hg])
                        for hl in range(2):
                            hrow = 256 * hg + 128 * hl
                            ost = sp.tile([128, cape], F32, tag="ost",
                                          bufs=4, name="ost")
                            for b, (bo, bs) in enumerate(blks):
                                o_ps = ps.tile([128, bs], F32, tag="o", bufs=2,
                                               name="o_ps")
                                for i in range(IT):
                                    nc.tensor.matmul(
                                        o_ps[:],
                                        w2_sb[:, i * 256 + hl * 128
                                              : i * 256 + hl * 128 + 128],
                                        h_sb[:, i * cape + bo
                                             : i * cape + bo + bs],
                                        start=(i == 0), stop=(i == IT - 1))
                                nc.vector.tensor_copy(ost[:, bo : bo + bs],
                                                      o_ps[:])
                            nc.sync.dma_start(
                                yd_d[hrow : hrow + 128, s0 : s0 + cape],
                                ost[:])

                # ============ shared expert, full IS, this core's 512 tokens
                x_sb = xp.tile([128, HT * tsh], BF16, tag="x", name="xsh_sb")
                nc.sync.dma_start(x_sb[:], xsh_d[:])
                hs_sb = hp.tile([128, IST * tsh], BF16, tag="h", name="hs_sb")
                for ig in range(IST):
                    w13_sb = wp.tile([128, HT * 256], BF16, tag="w13",
                                     name="sw13_sb")
                    nc.sync.dma_start(w13_sb[:], sw13_d[ig])
                    g_ps = ps.tile([128, tsh], F32, tag=f"g{ig % 2}",
                                   name="g_ps0")
                    u_ps = ps.tile([128, tsh], F32, tag=f"u{ig % 2}",
                                   name="u_ps0")
                    for gu in range(2):
                        p = g_ps if gu == 0 else u_ps
                        for h in range(HT):
                            nc.tensor.matmul(
                                p[:],
                                w13_sb[:, h * 256 + gu * 128
                                       : h * 256 + gu * 128 + 128],
                                x_sb[:, bass.ts(h, tsh)],
                                start=(h == 0), stop=(h == HT - 1))
                    silu_sb = sp.tile([128, 512], BF16, tag="silu", bufs=4,
                                      name="silu_sb")
                    nc.scalar.activation(silu_sb[:, :tsh], g_ps[:], AF.Silu)
                    nc.vector.tensor_mul(hs_sb[:, bass.ts(ig, tsh)], u_ps[:],
                                         silu_sb[:, :tsh])
                for hg in range(HT // 2):
                    w2_sb = w2p.tile([128, IST * 256], BF16, tag="w2",
                                     name="sw2_sb")
                    nc.sync.dma_start(w2_sb[:], sw2t_d[hg])
                    for hl in range(2):
                        hrow = 256 * hg + 128 * hl
                        o_ps = ps.tile([128, tsh], F32, tag="o", bufs=2,
                                       name="o_ps")
                        for i in range(IST):
                            nc.tensor.matmul(
                                o_ps[:],
                                w2_sb[:, i * 256 + hl * 128
                                      : i * 256 + hl * 128 + 128],
                                hs_sb[:, bass.ts(i, tsh)],
                                start=(i == 0), stop=(i == IST - 1))
                        ost = sp.tile([128, 512], F32, tag="ost", bufs=4,
                                      name="ost")
                        nc.vector.tensor_copy(ost[:, :tsh], o_ps[:])
                        nc.sync.dma_start(ys_d[hrow : hrow + 128, :],
                                          ost[:, :tsh])
    nc.compile()
    return nc


def prep_inputs2(hidden_states, gate_w, e_bias, w1, w3, w2, sw1, sw3, sw2,
                 slots=None, caps=None):
    x = np.asarray(hidden_states, np.float32)
    t_total = x.shape[0]
    tsh = t_total // N_CORES
    wfull = host_routing(x, np.asarray(gate_w, np.float32),
                         np.asarray(e_bias, np.float32))
    if slots is None:
        slots, caps = plan_slots(wfull)
    total_cap = sum(caps)
    xT = np.ascontiguousarray(x.T.astype(NPBF16))

    w1t = np.asarray(w1, np.float32).transpose(0, 2, 1).astype(NPBF16)
    w3t = np.asarray(w3, np.float32).transpose(0, 2, 1).astype(NPBF16)
    w13 = np.empty((E, H, IT, 256), NPBF16)
    w13[..., 0:128] = np.ascontiguousarray(w1t).reshape(E, H, IT, 128)
    w13[..., 128:256] = np.ascontiguousarray(w3t).reshape(E, H, IT, 128)
    # pack to the exact SBUF tile layout: [E, IT, 128p, HT*256]
    w13 = np.ascontiguousarray(
        w13.transpose(0, 2, 1, 3).reshape(E, IT, HT, 128, 256)
        .transpose(0, 1, 3, 2, 4).reshape(E, IT, 128, HT * 256))
    w2t = np.asarray(w2, np.float32).transpose(0, 2, 1).astype(NPBF16)
    # [E, I, H] -> [E, HT//2, 128p, IT*256]
    w2t = np.ascontiguousarray(
        w2t.reshape(E, IT, 128, HT // 2, 256)
        .transpose(0, 3, 2, 1, 4).reshape(E, HT // 2, 128, IT * 256))

    sw1t = np.ascontiguousarray(np.asarray(sw1, np.float32).T)  # [H, IS]
    sw3t = np.ascontiguousarray(np.asarray(sw3, np.float32).T)
    sw13 = np.empty((H, IST, 256), NPBF16)
    for ig in range(IST):
        sw13[:, ig, 0:128] = sw1t[:, 128 * ig : 128 * (ig + 1)]
        sw13[:, ig, 128:256] = sw3t[:, 128 * ig : 128 * (ig + 1)]
    # [H, IST, 256] -> [IST, 128p, HT*256]
    sw13 = np.ascontiguousarray(
        sw13.transpose(1, 0, 2).reshape(IST, HT, 128, 256)
        .transpose(0, 2, 1, 3).reshape(IST, 128, HT * 256))
    sw2t = np.asarray(sw2, np.float32).T.astype(NPBF16)  # [IS, H]
    # [IS, H] -> [HT//2, 128p, IST*256]
    sw2t = np.ascontiguousarray(
        sw2t.reshape(IST, 128, HT // 2, 256)
        .transpose(2, 1, 0, 3).reshape(HT // 2, 128, IST * 256))

    def pack_x(cols):  # [H, n] -> [128p, HT*n]
        n = cols.shape[1]
        return (cols.reshape(HT, 128, n).transpose(1, 0, 2)
                .reshape(128, HT * n))

    in_maps, scat = [], []
    for c in range(N_CORES):
        idxs, xsegs = [], []
        for j in range(EL):
            e_id = int(slots[j][c])
            idx = np.nonzero(wfull[:, e_id])[0]
            assert len(idx) <= caps[j], f"expert {e_id}: {len(idx)} > {caps[j]}"
            pad = caps[j] - len(idx)
            tok = np.concatenate([idx, np.zeros(pad, np.int64)])
            xsegs.append(pack_x(xT[:, tok]))
            idxs.append((e_id, idx))
        in_maps.append({
            "xs": np.ascontiguousarray(np.concatenate(xsegs, axis=1)),
            "xsh": np.ascontiguousarray(
                pack_x(xT[:, c * tsh : (c + 1) * tsh])),
            "w13": np.ascontiguousarray(w13[[slots[j][c] for j in range(EL)]]),
            "w2t": np.ascontiguousarray(w2t[[slots[j][c] for j in range(EL)]]),
            "sw13": sw13, "sw2t": sw2t,
        })
        scat.append(idxs)
    return in_maps, scat, wfull, caps


def combine2(results, scat, wfull, caps, t_total=T):
    tsh = t_total // N_CORES
    acc = np.zeros((H, t_total), np.float32)
    for c in range(N_CORES):
        acc[:, c * tsh : (c + 1) * tsh] = results[c]["ys"]
    for c in range(N_CORES):
        yd = results[c]["yd"]
        for j, (e_id, idx) in enumerate(scat[c]):
            s0 = sum(caps[:j])
            colv = wfull[idx, e_id].astype(np.float32)
            acc[:, idx] += yd[:, s0 : s0 + len(idx)] * colv[None, :]
    return np.ascontiguousarray(acc.T)


_NC_CACHE = {}


def run2(inputs):
    from concourse.bass_utils import run_bass_kernel_spmd
    x = np.asarray(inputs["hidden_states"], np.float32)
    wfull = host_routing(x, np.asarray(inputs["gate_w"], np.float32),
                         np.asarray(inputs["e_bias"], np.float32))
    slots, caps = plan_slots(wfull)
    key = (caps, x.shape[0])
    if key not in _NC_CACHE:
        _NC_CACHE[key] = build_nc2(caps, tsh=x.shape[0] // N_CORES)
    nc = _NC_CACHE[key]
    in_maps, scat, wfull, caps = prep_inputs2(**inputs, slots=slots, caps=caps)
    res = run_bass_kernel_spmd(nc, in_maps, core_ids=list(range(N_CORES)))
    return combine2(res.results, scat, wfull, caps, t_total=x.shape[0]), res


def kernel(**inputs) -> np.ndarray:
    out, _ = run2(inputs)
    return np.asarray(out, np.float32)
